# revision 1
# baseline (speedup 1.0000x reference)
"""Trainium2 Bass kernel for nn_Decoder (fc + 3-layer GRU + mask).

Strategy: data-parallel over batch B=32 across 8 cores (4 samples/core).
Per core, all compute in gate-major ("ghT") layout: gates/hidden on the
partition dim, (hidden-chunk, batch) in the free dim. The recurrent
matmul keeps w_hh chunks as the stationary operand (bf16 -> FWL weight
load) and the tiny h-vector as the moving operand, accumulating the K
contraction in PSUM. gx (input-gate projections) are precomputed per
16-step chunk as GEMMs from the previous layer's stored h sequence
(layer 0: from the chord embedding stream).
"""

import os
import sys
from contextlib import ExitStack

for _p in ("/opt/trn_rl_repo",):
    if _p not in sys.path:
        sys.path.insert(0, _p)

import numpy as np
import ml_dtypes

import concourse.bass as bass
import concourse.bacc as bacc
import concourse.mybir as mybir
import concourse.tile as tile
from concourse import bass_utils

BF = np.float16
F32 = np.float32
dt = mybir.dt

NCORES = 8
B, T = 32, 512
BS = B // NCORES          # 4 samples per core
TC = 16                   # time-steps per chunk
NCHUNK = T // TC          # 32
HID = 1024                # layer-0 input dim
H = 512                   # GRU hidden
G3 = 3 * H                # 1536
MCH = G3 // 128           # 12 gate-dim chunks (m): order r0..r3 z0..z3 n0..n3
KC = H // 128             # 4 hidden chunks
KC0 = HID // 128          # 8 input chunks for layer 0
FREE = KC * BS            # 16 = (hc, b) free layout used everywhere
REPEAT = 0                # extra timing-only layer-1 passes
ABLATE = ""               # timing experiments: "nogates", "nochain"
RDEV = 0                  # device-side timing repeats of a layer-1 pass

Sigmoid = mybir.ActivationFunctionType.Sigmoid
Tanh = mybir.ActivationFunctionType.Tanh
Relu = mybir.ActivationFunctionType.Relu
Alu = mybir.AluOpType


def _declare_io(nc):
    d = {}
    inp = lambda n, s, t: nc.dram_tensor(n, s, t, kind="ExternalInput").ap()
    d["chordT"] = inp("chordT", [KC0, 128, T, BS], dt.float16)
    d["zT"] = inp("zT", [2, 128, BS], dt.float16)
    d["fcwT"] = inp("fcwT", [2, 128, HID], dt.float16)
    d["fcb"] = inp("fcb", [KC0, 128, 1], dt.float32)
    d["wihT0"] = inp("wihT0", [KC0, 128, G3], dt.float16)
    d["wihT1"] = inp("wihT1", [KC, 128, G3], dt.float16)
    d["wihT2"] = inp("wihT2", [KC, 128, G3], dt.float16)
    d["whhT0"] = inp("whhT0", [KC, 128, G3], dt.float16)
    d["whhT1"] = inp("whhT1", [KC, 128, G3], dt.float16)
    d["whhT2"] = inp("whhT2", [KC, 128, G3], dt.float16)
    d["biasrep"] = inp("biasrep", [3, 128, MCH * BS], dt.float32)
    d["bhhn"] = inp("bhhn", [3, 128, FREE], dt.float32)
    d["iota"] = inp("iota", [128, T], dt.float32)
    d["seqrep"] = inp("seqrep", [128, BS], dt.float32)
    d["outT"] = nc.dram_tensor("outT", [KC, 128, T, BS], dt.float32,
                               kind="ExternalOutput").ap()
    return d


def _build_program(debug=False):
    nc = bacc.Bacc("TRN2", target_bir_lowering=False, debug=debug,
                   num_devices=NCORES)
    io = _declare_io(nc)

    with tile.TileContext(nc) as tc:
        _emit(tc, io)
    nc.compile()
    return nc


def _emit(tc, io):
    nc = tc.nc
    ctx = ExitStack()
    const = ctx.enter_context(tc.tile_pool(name="const", bufs=1))
    stream = ctx.enter_context(tc.tile_pool(name="stream", bufs=3))
    gxp = ctx.enter_context(tc.tile_pool(name="gxp", bufs=2))
    tmp = ctx.enter_context(tc.tile_pool(name="tmp", bufs=3))
    outp = ctx.enter_context(tc.tile_pool(name="outp", bufs=2))
    pgh_pool = ctx.enter_context(tc.tile_pool(name="pgh", bufs=4, space="PSUM"))
    pgx_pool = ctx.enter_context(tc.tile_pool(name="pgx", bufs=2, space="PSUM"))

    # ---- persistent SBUF tensors -------------------------------------
    wih = [const.tile([128, KC0, G3], dt.float16, tag="wih0", name="wih0"),
           const.tile([128, KC, G3], dt.float16, tag="wih1", name="wih1"),
           const.tile([128, KC, G3], dt.float16, tag="wih2", name="wih2")]
    whh = [const.tile([128, KC, G3], dt.float16, tag=f"whh{l}", name=f"whh{l}") for l in range(3)]
    hseq = [const.tile([128, T, FREE], dt.float16, tag=f"hseq{i}", name=f"hseq{i}") for i in range(2)]
    mask = const.tile([128, T, BS], dt.float32, tag="mask")
    gb = [const.tile([128, MCH * BS], dt.float32, tag=f"gb{l}", name=f"gb{l}") for l in range(3)]
    bhhn = const.tile([128, 3, FREE], dt.float32, tag="bhhn")
    fc_hT = const.tile([128, KC0, BS], dt.float16, tag="fchT")
    hfp = const.tile([128, FREE], dt.float32, tag="hfp")
    hbf = const.tile([128, TC, FREE], dt.float16, tag="hbf")

    # ---- load weights / constants ------------------------------------
    for kc in range(KC0):
        nc.sync.dma_start(wih[0][:, kc, :], io["wihT0"][kc])
    for l in (1, 2):
        for kc in range(KC):
            nc.sync.dma_start(wih[l][:, kc, :], io[f"wihT{l}"][kc])
    for l in range(3):
        for kc in range(KC):
            nc.sync.dma_start(whh[l][:, kc, :], io[f"whhT{l}"][kc])
        nc.sync.dma_start(gb[l][:], io["biasrep"][l])
        nc.sync.dma_start(bhhn[:, l, :], io["bhhn"][l])

    # ---- prologue: mask, fc, gB0 -------------------------------------
    with ExitStack() as pctx:
        psb = pctx.enter_context(tc.tile_pool(name="psb", bufs=2))

        iota_sb = psb.tile([128, T], dt.float32, tag="iota")
        seq_sb = psb.tile([128, BS], dt.float32, tag="seq")
        nc.sync.dma_start(iota_sb[:], io["iota"])
        nc.sync.dma_start(seq_sb[:], io["seqrep"])
        for b in range(BS):
            nc.vector.tensor_scalar(mask[:, :, b], iota_sb[:],
                                    seq_sb[:, b:b + 1], None, op0=Alu.is_lt)

        z_sb = psb.tile([128, 2, BS], dt.float16, tag="zsb")
        fcw_sb = psb.tile([128, 2, HID], dt.float16, tag="fcw")
        fcb_sb = psb.tile([128, KC0], dt.float32, tag="fcb")
        for kc in range(2):
            nc.sync.dma_start(z_sb[:, kc, :], io["zT"][kc])
            nc.sync.dma_start(fcw_sb[:, kc, :], io["fcwT"][kc])
        for hc in range(KC0):
            nc.sync.dma_start(fcb_sb[:, hc:hc + 1], io["fcb"][hc])
        for hc in range(KC0):
            pfc = pgx_pool.tile([128, BS], dt.float32, tag="pgx", name="pfc")
            for kc in range(2):
                nc.tensor.matmul(pfc[:], fcw_sb[:, kc, hc * 128:(hc + 1) * 128],
                                 z_sb[:, kc, :], start=(kc == 0), stop=(kc == 1))
            nc.scalar.activation(fc_hT[:, hc, :], pfc[:], Relu,
                                 bias=fcb_sb[:, hc:hc + 1], scale=1.0)
        # gB0 += w_ih0 @ fc_hT
        for m in range(MCH):
            pgb = pgx_pool.tile([128, BS], dt.float32, tag="pgx", name="pgb")
            for kc in range(KC0):
                nc.tensor.matmul(pgb[:], wih[0][:, kc, m * 128:(m + 1) * 128],
                                 fc_hT[:, kc, :], start=(kc == 0),
                                 stop=(kc == KC0 - 1))
            nc.vector.tensor_add(gb[0][:, m * BS:(m + 1) * BS],
                                 gb[0][:, m * BS:(m + 1) * BS], pgb[:])

    # ---- per-layer chunk loop ----------------------------------------
    passes = [0, 1, 2] + [1] * REPEAT + ([1] if RDEV else [])
    for pidx, l in enumerate(passes):
        in_dev_repeat = RDEV and pidx == len(passes) - 1
        rep_cm = tc.For_i(0, RDEV, 1, name="rep") if in_dev_repeat else None
        if rep_cm is not None:
            rep_cm.__enter__()
        kcl = KC0 if l == 0 else KC
        cur = hseq[l % 2]
        prev = hseq[(l - 1) % 2]
        is_last = pidx == len(passes) - 1 if REPEAT == 0 else (pidx == 2)
        nc.gpsimd.memset(hfp[:], 0.0)
        nc.gpsimd.memset(hbf[:, TC - 1, :], 0.0)

        hint = (mybir.EngineType.PE,)
        with tc.For_i(0, T, TC, hint_engines=hint, name=f"pass{pidx}") as i:
            # -- gx GEMM for this chunk --
            if l == 0:
                src = stream.tile([128, TC, KC0, BS], dt.float16, tag="src0")
                for kc in range(KC0):
                    nc.sync.dma_start(src[:, :, kc, :],
                                      io["chordT"][kc, :, bass.ds(i, TC), :])
            else:
                src = stream.tile([128, TC, KC, BS], dt.float16, tag="src12")
                if "nodma" not in ABLATE:
                    nc.sync.dma_start(src[:], prev[:, bass.ds(i, TC), :])
                else:
                    nc.gpsimd.memset(src[:, 0, 0, :], 0.0)

            gx = gxp.tile([128, MCH, TC, BS], dt.float32, tag="gx")
            if "nogemm" in ABLATE:
                nc.gpsimd.memset(gx[:, 0, 0, :], 0.0)
            for m in range(0 if "nogemm" in ABLATE else MCH):
                pgx = pgx_pool.tile([128, TC * BS], dt.float32, tag="pgx")
                for kc in range(kcl):
                    nc.tensor.matmul(
                        pgx[:], wih[l][:, kc, m * 128:(m + 1) * 128],
                        src[:, :, kc, :], start=(kc == 0), stop=(kc == kcl - 1))
                nc.vector.tensor_add(
                    gx[:, m, :, :],
                    pgx[:].rearrange("p (t b) -> p t b", t=TC),
                    gb[l][:, m * BS:(m + 1) * BS]
                    .rearrange("p (o b) -> p o b", o=1).broadcast_to([128, TC, BS]))

            if l == 2 and ABLATE != "nogates":
                mch = stream.tile([128, TC, BS], dt.float32, tag="maskch")
                nc.sync.dma_start(mch[:], mask[:, bass.ds(i, TC), :])
                osb = outp.tile([128, KC, TC, BS], dt.float32, tag="osb")

            # -- TC recurrence steps --
            for s in range(0 if "nosteps" in ABLATE else TC):
                sp = (s - 1) % TC          # previous step's h slot
                if ABLATE == "nochain":
                    sp = TC - 1            # constant rhs: breaks serial chain
                pgh = pgh_pool.tile([128, MCH * BS], dt.float32, tag="pgh")
                for m in range(MCH):
                    for kc in range(KC):
                        nc.tensor.matmul(
                            pgh[:, m * BS:(m + 1) * BS],
                            whh[l][:, kc, m * 128:(m + 1) * 128],
                            hbf[:, sp, kc * BS:(kc + 1) * BS],
                            start=(kc == 0), stop=(kc == KC - 1))
                if ABLATE == "nogates":
                    continue
                arz = tmp.tile([128, 2 * FREE], dt.float32, tag="arz")
                nc.vector.tensor_add(arz[:], pgh[:, 0:2 * FREE],
                                     gx[:, 0:2 * KC, s, :])
                rz = tmp.tile([128, 2 * FREE], dt.float32, tag="rz")
                nc.scalar.activation(rz[:], arz[:], Sigmoid)
                t1 = tmp.tile([128, FREE], dt.float32, tag="t1")
                nc.vector.tensor_add(t1[:], pgh[:, 2 * FREE:3 * FREE],
                                     bhhn[:, l, :])
                rn = tmp.tile([128, FREE], dt.float32, tag="rn")
                nc.vector.tensor_mul(rn[:], t1[:], rz[:, 0:FREE])
                aN = tmp.tile([128, FREE], dt.float32, tag="aN")
                nc.vector.tensor_add(aN[:], rn[:], gx[:, 2 * KC:3 * KC, s, :])
                n = tmp.tile([128, FREE], dt.float32, tag="n")
                nc.scalar.activation(n[:], aN[:], Tanh)
                d = tmp.tile([128, FREE], dt.float32, tag="d")
                nc.vector.tensor_sub(d[:], hfp[:], n[:])
                zd = tmp.tile([128, FREE], dt.float32, tag="zd")
                nc.vector.tensor_mul(zd[:], rz[:, FREE:2 * FREE], d[:])
                nc.vector.tensor_add(hfp[:], n[:], zd[:])
                nc.vector.tensor_copy(hbf[:, s, :], hfp[:])
                if l == 2 and ABLATE != "nogates":
                    nc.vector.tensor_mul(
                        osb[:, :, s, :],
                        hfp[:].rearrange("p (h b) -> p h b", h=KC),
                        mch[:, s:s + 1, :].broadcast_to([128, KC, BS]))

            if l < 2:
                if "nodma" not in ABLATE:
                    nc.sync.dma_start(cur[:, bass.ds(i, TC), :], hbf[:])
            elif ABLATE != "nogates":
                for hc in range(KC):
                    nc.sync.dma_start(io["outT"][hc, :, bass.ds(i, TC), :],
                                      osb[:, hc, :, :])
        if rep_cm is not None:
            rep_cm.__exit__(None, None, None)
    ctx.close()


_CACHE = {}


def _get_program():
    if "nc" not in _CACHE:
        _CACHE["nc"] = _build_program()
    return _CACHE["nc"]


def _prep_shared(fc_w, fc_b, ws):
    """Host layout prep for the replicated weights (shared by all cores)."""
    sh = {}
    sh["fcwT"] = np.ascontiguousarray(
        fc_w.T.reshape(2, 128, HID)).astype(BF)
    sh["fcb"] = np.ascontiguousarray(fc_b.reshape(KC0, 128, 1)).astype(F32)
    for l in range(3):
        w_ih, w_hh, b_ih, b_hh = ws[l]
        kcl = KC0 if l == 0 else KC
        sh[f"wihT{l}"] = np.ascontiguousarray(
            w_ih.T.reshape(kcl, 128, G3)).astype(BF)
        sh[f"whhT{l}"] = np.ascontiguousarray(
            w_hh.T.reshape(KC, 128, G3)).astype(BF)
    br = np.zeros((3, 128, MCH, BS), F32)
    bn = np.zeros((3, 128, FREE), F32)
    for l in range(3):
        _, _, b_ih, b_hh = ws[l]
        bi = b_ih.reshape(MCH, 128)
        bh = b_hh.reshape(MCH, 128)
        v = bi.copy()
        v[:2 * KC] += bh[:2 * KC]          # r,z gates absorb b_hh
        br[l] = v.T[:, :, None]
        bn[l] = np.repeat(bh[2 * KC:].T[:, :, None], BS, axis=2).reshape(128, FREE)
    sh["biasrep"] = br.reshape(3, 128, MCH * BS)
    sh["bhhn"] = bn
    sh["iota"] = np.broadcast_to(
        np.arange(T, dtype=F32)[None, :], (128, T)).copy()
    return sh


def kernel(z, seq_lens, chord_embedding, fc_w, fc_b,
           w_ih0, w_hh0, b_ih0, b_hh0,
           w_ih1, w_hh1, b_ih1, b_hh1,
           w_ih2, w_hh2, b_ih2, b_hh2):
    z = np.asarray(z, F32)
    chord = np.asarray(chord_embedding, F32)
    seq = np.asarray(seq_lens)
    ws = [(np.asarray(w_ih0, F32), np.asarray(w_hh0, F32),
           np.asarray(b_ih0, F32), np.asarray(b_hh0, F32)),
          (np.asarray(w_ih1, F32), np.asarray(w_hh1, F32),
           np.asarray(b_ih1, F32), np.asarray(b_hh1, F32)),
          (np.asarray(w_ih2, F32), np.asarray(w_hh2, F32),
           np.asarray(b_ih2, F32), np.asarray(b_hh2, F32))]

    in_maps = _make_in_maps(z, seq, chord, np.asarray(fc_w, F32),
                            np.asarray(fc_b, F32), ws)
    res = _execute(in_maps)
    return _assemble(res.results)


def _make_in_maps(z, seq, chord, fc_w, fc_b, ws):
    sh = _prep_shared(fc_w, fc_b, ws)
    in_maps = []
    for c in range(NCORES):
        bs = slice(c * BS, (c + 1) * BS)
        m = dict(sh)
        m["chordT"] = np.ascontiguousarray(
            (chord[bs].transpose(2, 1, 0) / 100.0)
            .reshape(KC0, 128, T, BS)).astype(BF)
        m["zT"] = np.ascontiguousarray(
            z[bs].T.reshape(2, 128, BS)).astype(BF)
        m["seqrep"] = np.broadcast_to(
            seq[bs].astype(F32)[None, :], (128, BS)).copy()
        in_maps.append(m)
    return in_maps


def _execute(in_maps, **kw):
    nc = _get_program()
    return bass_utils.run_bass_kernel_spmd(nc, in_maps, list(range(NCORES)), **kw)


def _assemble(results):
    out = np.empty((B, T, H), F32)
    for c in range(NCORES):
        outT = np.asarray(results[c]["outT"])       # [KC,128,T,BS]
        out[c * BS:(c + 1) * BS] = (
            outT.transpose(3, 2, 0, 1).reshape(BS, T, H))
    return out



# revision 8
# speedup vs baseline: 1.0749x; 1.0749x over previous
"""Trainium2 Bass kernel for nn_Decoder (fc + 3-layer GRU + mask).

Strategy: data-parallel over batch B=32 across 8 cores (4 samples/core).
Per core, all compute in gate-major layout: gates/hidden on the
partition dim, (hidden-chunk, batch) in the free dim.

v2: the three GRU layers are processed as a chunk-level wavefront on
each core — round r runs (layer0, chunk r), (layer1, chunk r-1),
(layer2, chunk r-2). The three 16-step recurrences in a round are
independent, so the Tile scheduler overlaps one layer's PE matmuls
with another layer's serial DVE/ACT gate chain. w_hh is stored fp8e4m3
(scaled x64; the 1/64 folds into the activation `scale` input), which
halves the LDWEIGHTS time that dominates the recurrence.
"""

import os
import sys
from contextlib import ExitStack

for _p in ("/opt/trn_rl_repo",):
    if _p not in sys.path:
        sys.path.insert(0, _p)

import numpy as np
import ml_dtypes

import concourse.bass as bass
import concourse.bacc as bacc
import concourse.mybir as mybir
import concourse.tile as tile
from concourse import bass_utils

BF = np.float16
F32 = np.float32
dt = mybir.dt

NCORES = 8
B, T = 32, 512
BS = B // NCORES          # 4 samples per core
TC = 16                   # time-steps per chunk
NCHUNK = T // TC          # 32
HID = 1024                # layer-0 input dim
H = 512                   # GRU hidden
G3 = 3 * H                # 1536
MCH = G3 // 128           # 12 gate-dim chunks (m): order r0..r3 z0..z3 n0..n3
KC = H // 128             # 4 hidden chunks
KC0 = HID // 128          # 8 input chunks for layer 0
FREE = KC * BS            # 16 = (hc, b) free layout used everywhere
RDEV = 0                  # device-side timing repeats of the main loop
WSCALE = 64.0             # gate-path scale folded into activations
# r,z recurrent weights in fp8 e4m3 (halves their LDWEIGHTS time; error is
# bounded through the sigmoids); n-gate weights stay fp16 (feed h directly).
WRZ_DT = dt.float8e4
WRZ_NP = ml_dtypes.float8_e4m3

Sigmoid = mybir.ActivationFunctionType.Sigmoid
Tanh = mybir.ActivationFunctionType.Tanh
Relu = mybir.ActivationFunctionType.Relu
Alu = mybir.AluOpType


def _declare_io(nc):
    d = {}
    inp = lambda n, s, t: nc.dram_tensor(n, s, t, kind="ExternalInput").ap()
    d["chordT"] = inp("chordT", [KC0, 128, T, BS], dt.float16)
    d["zT"] = inp("zT", [2, 128, BS], dt.float16)
    d["fcwT"] = inp("fcwT", [2, 128, HID], dt.float16)
    d["fcb"] = inp("fcb", [KC0, 128, 1], dt.float32)
    d["wihT0"] = inp("wihT0", [KC0, 128, G3], dt.float16)
    d["wihT1"] = inp("wihT1", [KC, 128, G3], dt.float16)
    d["wihT2"] = inp("wihT2", [KC, 128, G3], dt.float16)
    for l in range(3):
        d[f"whhrzT{l}"] = inp(f"whhrzT{l}", [KC, 128, 2 * H], WRZ_DT)
        d[f"whhnT{l}"] = inp(f"whhnT{l}", [KC, 128, H], dt.float16)
    d["biasrep"] = inp("biasrep", [3, 128, MCH * BS], dt.float32)
    d["bhhn"] = inp("bhhn", [3, 128, FREE], dt.float32)
    d["iota"] = inp("iota", [128, T], dt.float32)
    d["seqrep"] = inp("seqrep", [128, BS], dt.float32)
    d["outT"] = nc.dram_tensor("outT", [KC, 128, T, BS], dt.float32,
                               kind="ExternalOutput").ap()
    return d


def _build_program(debug=False):
    nc = bacc.Bacc("TRN2", target_bir_lowering=False, debug=debug,
                   num_devices=NCORES)
    io = _declare_io(nc)

    with tile.TileContext(nc) as tc:
        _emit(tc, io)
    nc.compile()
    return nc


class _State:
    pass


def _emit(tc, io):
    nc = tc.nc
    ctx = ExitStack()
    st = _State()
    const = ctx.enter_context(tc.tile_pool(name="const", bufs=1))
    stream = ctx.enter_context(tc.tile_pool(name="stream", bufs=3))
    gxp = ctx.enter_context(tc.tile_pool(name="gxp", bufs=4))
    tmp = ctx.enter_context(tc.tile_pool(name="tmp", bufs=3))
    outp = ctx.enter_context(tc.tile_pool(name="outp", bufs=2))
    st.stream, st.gxp, st.tmp, st.outp = stream, gxp, tmp, outp
    st.pgh_pool = ctx.enter_context(tc.tile_pool(name="pgh", bufs=5, space="PSUM"))
    st.pgx_pool = ctx.enter_context(tc.tile_pool(name="pgx", bufs=3, space="PSUM"))

    # ---- persistent SBUF tensors -------------------------------------
    st.wih = [const.tile([128, KC0, G3], dt.float16, tag="wih0", name="wih0"),
              const.tile([128, KC, G3], dt.float16, tag="wih1", name="wih1"),
              const.tile([128, KC, G3], dt.float16, tag="wih2", name="wih2")]
    st.whhrz = [const.tile([128, KC, 2 * H], WRZ_DT, tag=f"whhrz{l}",
                           name=f"whhrz{l}") for l in range(3)]
    st.whhn = [const.tile([128, KC, H], dt.float16, tag=f"whhn{l}",
                          name=f"whhn{l}") for l in range(3)]
    st.mask = const.tile([128, T, BS], dt.float32, tag="mask")
    st.gb = [const.tile([128, MCH * BS], dt.float32, tag=f"gb{l}", name=f"gb{l}")
             for l in range(3)]
    st.bhhn = const.tile([128, 3, FREE], dt.float32, tag="bhhn")
    st.fc_hT = const.tile([128, KC0, BS], dt.float16, tag="fchT")
    st.hfp = [const.tile([128, FREE], dt.float32, tag=f"hfp{l}", name=f"hfp{l}")
              for l in range(3)]
    st.hbf = [const.tile([128, TC, FREE], dt.float16, tag=f"hbf{l}", name=f"hbf{l}")
              for l in range(3)]

    # ---- load weights / constants ------------------------------------
    for kc in range(KC0):
        nc.sync.dma_start(st.wih[0][:, kc, :], io["wihT0"][kc])
    for l in (1, 2):
        for kc in range(KC):
            nc.sync.dma_start(st.wih[l][:, kc, :], io[f"wihT{l}"][kc])
    for l in range(3):
        for kc in range(KC):
            nc.sync.dma_start(st.whhrz[l][:, kc, :], io[f"whhrzT{l}"][kc])
            nc.sync.dma_start(st.whhn[l][:, kc, :], io[f"whhnT{l}"][kc])
        nc.sync.dma_start(st.gb[l][:], io["biasrep"][l])
        nc.sync.dma_start(st.bhhn[:, l, :], io["bhhn"][l])

    # ---- prologue: mask, fc, gB0, zero state -------------------------
    with ExitStack() as pctx:
        psb = pctx.enter_context(tc.tile_pool(name="psb", bufs=2))

        iota_sb = psb.tile([128, T], dt.float32, tag="iota")
        seq_sb = psb.tile([128, BS], dt.float32, tag="seq")
        nc.sync.dma_start(iota_sb[:], io["iota"])
        nc.sync.dma_start(seq_sb[:], io["seqrep"])
        for b in range(BS):
            nc.vector.tensor_scalar(st.mask[:, :, b], iota_sb[:],
                                    seq_sb[:, b:b + 1], None, op0=Alu.is_lt)

        z_sb = psb.tile([128, 2, BS], dt.float16, tag="zsb")
        fcw_sb = psb.tile([128, 2, HID], dt.float16, tag="fcw")
        fcb_sb = psb.tile([128, KC0], dt.float32, tag="fcb")
        for kc in range(2):
            nc.sync.dma_start(z_sb[:, kc, :], io["zT"][kc])
            nc.sync.dma_start(fcw_sb[:, kc, :], io["fcwT"][kc])
        for hc in range(KC0):
            nc.sync.dma_start(fcb_sb[:, hc:hc + 1], io["fcb"][hc])
        for hc in range(KC0):
            pfc = st.pgx_pool.tile([128, BS], dt.float32, tag="pgx", name="pfc")
            for kc in range(2):
                nc.tensor.matmul(pfc[:], fcw_sb[:, kc, hc * 128:(hc + 1) * 128],
                                 z_sb[:, kc, :], start=(kc == 0), stop=(kc == 1))
            nc.scalar.activation(st.fc_hT[:, hc, :], pfc[:], Relu,
                                 bias=fcb_sb[:, hc:hc + 1], scale=1.0)
        # gB0 += w_ih0 @ fc_hT   (fc output is time-constant -> fold into bias)
        for m in range(MCH):
            pgb = st.pgx_pool.tile([128, BS], dt.float32, tag="pgx", name="pgb")
            for kc in range(KC0):
                nc.tensor.matmul(pgb[:], st.wih[0][:, kc, m * 128:(m + 1) * 128],
                                 st.fc_hT[:, kc, :], start=(kc == 0),
                                 stop=(kc == KC0 - 1))
            nc.vector.tensor_add(st.gb[0][:, m * BS:(m + 1) * BS],
                                 st.gb[0][:, m * BS:(m + 1) * BS], pgb[:])

    for l in range(3):
        nc.gpsimd.memset(st.hfp[l][:], 0.0)
        nc.gpsimd.memset(st.hbf[l][:, TC - 1, :], 0.0)

    # ---- wavefront rounds --------------------------------------------
    # Round r: (L0, chunk r), (L1, chunk r-1), (L2, chunk r-2).
    # chordT is host-rolled left by 2 chunks so that inside the hardware
    # loop (which runs L2's chunk offset i) L0's chunk i+2*TC sits at
    # offset i; peeled rounds 0/1 read chunks 0/1 at offsets 30/31*TC.
    def _sl(ap_time_dim, pos, extra_lead=0):
        # slicing helper: pos is an int (peeled) or a loop var (ds)
        if isinstance(pos, int):
            o = (pos + extra_lead) * TC % T
            return slice(o, o + TC)
        return bass.ds(pos, TC)

    def emit_round(r, pos):
        # pos: compile-time int round index, or loop var (= L2 chunk offset)
        l0 = isinstance(pos, int) and 0 <= r < NCHUNK or not isinstance(pos, int)
        l1 = isinstance(pos, int) and 0 <= r - 1 < NCHUNK or not isinstance(pos, int)
        l2 = isinstance(pos, int) and 0 <= r - 2 < NCHUNK or not isinstance(pos, int)

        # stage previous layers' chunk outputs before they are overwritten
        if l1:
            src1 = stream.tile([128, TC, KC, BS], dt.float16, tag="src1")
            nc.sync.dma_start(src1[:], st.hbf[0][:])
        if l2:
            src2 = stream.tile([128, TC, KC, BS], dt.float16, tag="src2")
            nc.sync.dma_start(src2[:], st.hbf[1][:])

        if l0:
            src0 = stream.tile([128, TC, KC0, BS], dt.float16, tag="src0")
            if isinstance(pos, int):
                tsl = _sl(None, r, extra_lead=-2)  # chunk r lives at (r-2)%32
            else:
                tsl = bass.ds(pos, TC)
            for kc in range(KC0):
                nc.sync.dma_start(src0[:, :, kc, :], io["chordT"][kc, :, tsl, :])
            emit_chunk(0, src0, KC0, None, None)
        if l1:
            emit_chunk(1, src1, KC, None, None)
        if l2:
            if isinstance(pos, int):
                osl = _sl(None, r - 2)
            else:
                osl = bass.ds(pos, TC)
            emit_chunk(2, src2, KC, osl, pos)

    def emit_chunk(l, src, kcl, osl, pos):
        # -- gx GEMM for this chunk --
        gx = gxp.tile([128, MCH, TC, BS], dt.float32, tag="gx")
        for m in range(MCH):
            pgx = st.pgx_pool.tile([128, TC * BS], dt.float32, tag="pgx")
            for kc in range(kcl):
                nc.tensor.matmul(
                    pgx[:], st.wih[l][:, kc, m * 128:(m + 1) * 128],
                    src[:, :, kc, :], start=(kc == 0), stop=(kc == kcl - 1))
            nc.vector.tensor_add(
                gx[:, m, :, :],
                pgx[:].rearrange("p (t b) -> p t b", t=TC),
                st.gb[l][:, m * BS:(m + 1) * BS]
                .rearrange("p (o b) -> p o b", o=1).broadcast_to([128, TC, BS]))

        if l == 2:
            mch = stream.tile([128, TC, BS], dt.float32, tag="maskch")
            nc.sync.dma_start(mch[:], st.mask[:, osl, :])
            osb = outp.tile([128, KC, TC, BS], dt.float32, tag="osb")

        # -- TC recurrence steps --
        inv = 1.0 / WSCALE
        hfp, hbf = st.hfp[l], st.hbf[l]
        whhrz, whhn = st.whhrz[l], st.whhn[l]
        for s in range(TC):
            sp = (s - 1) % TC          # previous step's h slot
            pgh = st.pgh_pool.tile([128, MCH * BS], dt.float32, tag="pgh")
            for m in range(MCH):
                wsl = (whhrz[:, :, m * 128:(m + 1) * 128] if m < 2 * KC
                       else whhn[:, :, (m - 2 * KC) * 128:(m - 2 * KC + 1) * 128])
                for kc in range(KC):
                    nc.tensor.matmul(
                        pgh[:, m * BS:(m + 1) * BS],
                        wsl[:, kc, :],
                        hbf[:, sp, kc * BS:(kc + 1) * BS],
                        start=(kc == 0), stop=(kc == KC - 1))
            arz = tmp.tile([128, 2 * FREE], dt.float32, tag="arz")
            nc.vector.tensor_add(arz[:], pgh[:, 0:2 * FREE],
                                 gx[:, 0:2 * KC, s, :])
            rz = tmp.tile([128, 2 * FREE], dt.float32, tag="rz")
            nc.scalar.activation(rz[:], arz[:], Sigmoid, scale=inv)
            t1 = tmp.tile([128, FREE], dt.float32, tag="t1")
            nc.vector.tensor_add(t1[:], pgh[:, 2 * FREE:3 * FREE],
                                 st.bhhn[:, l, :])
            rn = tmp.tile([128, FREE], dt.float32, tag="rn")
            nc.vector.tensor_mul(rn[:], t1[:], rz[:, 0:FREE])
            aN = tmp.tile([128, FREE], dt.float32, tag="aN")
            nc.vector.tensor_add(aN[:], rn[:], gx[:, 2 * KC:3 * KC, s, :])
            n = tmp.tile([128, FREE], dt.float32, tag="n")
            nc.scalar.activation(n[:], aN[:], Tanh, scale=inv)
            d = tmp.tile([128, FREE], dt.float32, tag="d")
            nc.vector.tensor_sub(d[:], hfp[:], n[:])
            zd = tmp.tile([128, FREE], dt.float32, tag="zd")
            nc.vector.tensor_mul(zd[:], rz[:, FREE:2 * FREE], d[:])
            nc.vector.tensor_add(hfp[:], n[:], zd[:])
            nc.vector.tensor_copy(hbf[:, s, :], hfp[:])
            if l == 2:
                nc.vector.tensor_mul(
                    osb[:, :, s, :],
                    hfp[:].rearrange("p (h b) -> p h b", h=KC),
                    mch[:, s:s + 1, :].broadcast_to([128, KC, BS]))

        if l == 2:
            for hc in range(KC):
                nc.sync.dma_start(io["outT"][hc, :, osl, :],
                                  osb[:, hc, :, :])

    hint = (mybir.EngineType.PE,)
    # peeled fill rounds 0,1
    emit_round(0, 0)
    emit_round(1, 1)
    # steady-state rounds 2..31 as a hardware loop over L2's chunk offset
    with tc.For_i(0, (NCHUNK - 2) * TC, TC, hint_engines=hint, name="main") as i:
        emit_round(None, i)
    # peeled drain rounds 32,33
    emit_round(NCHUNK, NCHUNK)
    emit_round(NCHUNK + 1, NCHUNK + 1)

    # optional timing-only repeat of the steady-state loop
    if RDEV:
        with tc.For_i(0, RDEV, 1, name="rep"):
            with tc.For_i(0, (NCHUNK - 2) * TC, TC, hint_engines=hint,
                          name="mainrep") as i:
                emit_round(None, i)
    ctx.close()


_CACHE = {}


def _get_program():
    if "nc" not in _CACHE:
        _CACHE["nc"] = _build_program()
    return _CACHE["nc"]


def _prep_shared(fc_w, fc_b, ws):
    """Host layout prep for the replicated weights (shared by all cores)."""
    sh = {}
    sh["fcwT"] = np.ascontiguousarray(
        fc_w.T.reshape(2, 128, HID)).astype(BF)
    sh["fcb"] = np.ascontiguousarray(fc_b.reshape(KC0, 128, 1)).astype(F32)
    for l in range(3):
        w_ih, w_hh, b_ih, b_hh = ws[l]
        kcl = KC0 if l == 0 else KC
        sh[f"wihT{l}"] = np.ascontiguousarray(
            (w_ih.T * WSCALE).reshape(kcl, 128, G3)).astype(BF)
        whT = (w_hh.T * WSCALE).reshape(KC, 128, G3)
        sh[f"whhrzT{l}"] = np.ascontiguousarray(
            whT[:, :, :2 * H]).astype(WRZ_NP)
        sh[f"whhnT{l}"] = np.ascontiguousarray(whT[:, :, 2 * H:]).astype(BF)
    br = np.zeros((3, 128, MCH, BS), F32)
    bn = np.zeros((3, 128, FREE), F32)
    for l in range(3):
        _, _, b_ih, b_hh = ws[l]
        bi = b_ih.reshape(MCH, 128)
        bh = b_hh.reshape(MCH, 128)
        v = bi.copy()
        v[:2 * KC] += bh[:2 * KC]          # r,z gates absorb b_hh
        br[l] = WSCALE * v.T[:, :, None]
        bn[l] = WSCALE * np.repeat(
            bh[2 * KC:].T[:, :, None], BS, axis=2).reshape(128, FREE)
    sh["biasrep"] = br.reshape(3, 128, MCH * BS)
    sh["bhhn"] = bn
    sh["iota"] = np.broadcast_to(
        np.arange(T, dtype=F32)[None, :], (128, T)).copy()
    return sh


def kernel(z, seq_lens, chord_embedding, fc_w, fc_b,
           w_ih0, w_hh0, b_ih0, b_hh0,
           w_ih1, w_hh1, b_ih1, b_hh1,
           w_ih2, w_hh2, b_ih2, b_hh2):
    z = np.asarray(z, F32)
    chord = np.asarray(chord_embedding, F32)
    seq = np.asarray(seq_lens)
    ws = [(np.asarray(w_ih0, F32), np.asarray(w_hh0, F32),
           np.asarray(b_ih0, F32), np.asarray(b_hh0, F32)),
          (np.asarray(w_ih1, F32), np.asarray(w_hh1, F32),
           np.asarray(b_ih1, F32), np.asarray(b_hh1, F32)),
          (np.asarray(w_ih2, F32), np.asarray(w_hh2, F32),
           np.asarray(b_ih2, F32), np.asarray(b_hh2, F32))]

    in_maps = _make_in_maps(z, seq, chord, np.asarray(fc_w, F32),
                            np.asarray(fc_b, F32), ws)
    res = _execute(in_maps)
    return _assemble(res.results)


def _make_in_maps(z, seq, chord, fc_w, fc_b, ws):
    sh = _prep_shared(fc_w, fc_b, ws)
    in_maps = []
    for c in range(NCORES):
        bs = slice(c * BS, (c + 1) * BS)
        m = dict(sh)
        chT = np.ascontiguousarray(
            (chord[bs].transpose(2, 1, 0) / 100.0)
            .reshape(KC0, 128, T, BS)).astype(BF)
        # roll time left by 2 chunks: chunk c sits at offset (c-2) % NCHUNK
        m["chordT"] = np.ascontiguousarray(np.roll(chT, -2 * TC, axis=2))
        m["zT"] = np.ascontiguousarray(
            z[bs].T.reshape(2, 128, BS)).astype(BF)
        m["seqrep"] = np.broadcast_to(
            seq[bs].astype(F32)[None, :], (128, BS)).copy()
        in_maps.append(m)
    return in_maps


def _execute(in_maps, **kw):
    nc = _get_program()
    return bass_utils.run_bass_kernel_spmd(nc, in_maps, list(range(NCORES)), **kw)


def _assemble(results):
    out = np.empty((B, T, H), F32)
    for c in range(NCORES):
        outT = np.asarray(results[c]["outT"])       # [KC,128,T,BS]
        out[c * BS:(c + 1) * BS] = (
            outT.transpose(3, 2, 0, 1).reshape(BS, T, H))
    return out


# revision 28
# speedup vs baseline: 1.6478x; 1.5330x over previous
"""Trainium2 Bass kernel for nn_Decoder (fc + 3-layer GRU + mask).

Strategy: data-parallel over batch B=32 across 8 cores (4 samples/core).
Per core, all compute in gate-major layout: gates/hidden on the
partition dim, (hidden-chunk, batch) in the free dim.

v3: chunk-level wavefront over the three GRU layers (round r runs
(L0, chunk r), (L1, chunk r-1), (L2, chunk r-2)) with the recurrence
steps round-robin across layers, so each layer's serial gate chain
hides behind the other layers' LDWEIGHTS-bound matmul bursts. The r,z
recurrent weights are fp8e4m3 (scaled x64, descaled via the activation
`scale` input); their input-projection gx lives in PSUM and the
recurrence matmuls accumulate straight onto it, so the sigmoid reads
PSUM directly. All gate biases enter via tiny K=1/K=4 matmuls instead
of vector ops. h is carried in fp16 only. The fc layer is folded into
the chord stream on the host (it is time-constant).
"""

import os
import sys
from contextlib import ExitStack

for _p in ("/opt/trn_rl_repo",):
    if _p not in sys.path:
        sys.path.insert(0, _p)

import numpy as np
import ml_dtypes

import concourse.bass as bass
import concourse.bacc as bacc
import concourse.mybir as mybir
import concourse.tile as tile
from concourse import bass_utils

BF = np.float16
F32 = np.float32
dt = mybir.dt

NCORES = 8
B, T = 32, 512
BS = B // NCORES          # 4 samples per core
TC = 16                   # time-steps per chunk
NCHUNK = T // TC          # 32
HID = 1024                # layer-0 input dim
H = 512                   # GRU hidden
G3 = 3 * H                # 1536
MCH = G3 // 128           # 12 gate-dim chunks (m): order r0..r3 z0..z3 n0..n3
KC = H // 128             # 4 hidden chunks
KC0 = HID // 128          # 8 input chunks for layer 0
FREE = KC * BS            # 16 = (hc, b) free layout used everywhere
PBANK = 512               # one PSUM bank in fp32 words
RDEV = 0                  # device-side timing repeats of the main loop
ABLATE = ""               # timing experiments: "nochain", "nogates"
LOOPN = NCHUNK - 2        # main-loop iterations (30; smaller = timing expt)
PEEL = True               # emit fill/drain rounds (False = timing expt)
WSCALE = 64.0             # gate-path scale folded into activations
# r,z recurrent weights in fp8 e4m3 (halves their LDWEIGHTS time; error is
# bounded through the sigmoids); n-gate weights stay fp16 (feed h directly).
WRZ_DT = dt.float8e4
WRZ_NP = ml_dtypes.float8_e4m3

Sigmoid = mybir.ActivationFunctionType.Sigmoid
Tanh = mybir.ActivationFunctionType.Tanh
Alu = mybir.AluOpType


def _declare_io(nc):
    d = {}
    inp = lambda n, s, t: nc.dram_tensor(n, s, t, kind="ExternalInput").ap()
    d["chordT"] = inp("chordT", [KC0, 128, T, BS], dt.float16)
    d["wihT0"] = inp("wihT0", [KC0, 128, G3], dt.float16)
    d["wihT1"] = inp("wihT1", [KC, 128, G3], dt.float16)
    d["wihT2"] = inp("wihT2", [KC, 128, G3], dt.float16)
    for l in range(3):
        d[f"whhrzT{l}"] = inp(f"whhrzT{l}", [KC, 128, 2 * H], WRZ_DT)
        d[f"whhnT{l}"] = inp(f"whhnT{l}", [KC, 128, H], dt.float16)
    d["gbstat"] = inp("gbstat", [3, 1, G3], dt.float16)
    d["bhhstat"] = inp("bhhstat", [3, 4, 128], dt.float16)
    d["sel4"] = inp("sel4", [4, FREE], dt.float16)
    d["ones64"] = inp("ones64", [1, TC * BS], dt.float16)
    d["iota"] = inp("iota", [128, T], dt.float32)
    d["seqrep"] = inp("seqrep", [128, BS], dt.float32)
    d["outT"] = nc.dram_tensor("outT", [KC, 128, T, BS], dt.float32,
                               kind="ExternalOutput").ap()
    return d


def _build_program(debug=False):
    nc = bacc.Bacc("TRN2", target_bir_lowering=False, debug=debug,
                   num_devices=NCORES)
    io = _declare_io(nc)

    with tile.TileContext(nc) as tc:
        _emit(tc, io)
    nc.compile()
    return nc


class _State:
    pass


def _emit(tc, io):
    nc = tc.nc
    ctx = ExitStack()
    st = _State()
    const = ctx.enter_context(tc.tile_pool(name="const", bufs=1))
    stream = ctx.enter_context(tc.tile_pool(name="stream", bufs=3))
    gxnp = ctx.enter_context(tc.tile_pool(name="gxnp", bufs=4))
    tmp = ctx.enter_context(tc.tile_pool(name="tmp", bufs=6))
    outp = ctx.enter_context(tc.tile_pool(name="outp", bufs=2))
    # every PSUM tile is padded to a full bank so start=True's whole-bank
    # has_written clear can never touch a neighbouring live tile
    prz_pool = ctx.enter_context(tc.tile_pool(name="prz", bufs=3, space="PSUM"))
    pghn_pool = ctx.enter_context(tc.tile_pool(name="pghn", bufs=3, space="PSUM"))
    pgx_pool = ctx.enter_context(tc.tile_pool(name="pgx", bufs=2, space="PSUM"))

    # ---- persistent SBUF tensors -------------------------------------
    st.wih = [const.tile([128, KC0, G3], dt.float16, tag="wih0", name="wih0"),
              const.tile([128, KC, G3], dt.float16, tag="wih1", name="wih1"),
              const.tile([128, KC, G3], dt.float16, tag="wih2", name="wih2")]
    st.whhrz = [const.tile([128, KC, 2 * H], WRZ_DT, tag=f"whhrz{l}",
                           name=f"whhrz{l}") for l in range(3)]
    st.whhn = [const.tile([128, KC, H], dt.float16, tag=f"whhn{l}",
                          name=f"whhn{l}") for l in range(3)]
    st.mask = const.tile([128, T, BS], dt.float32, tag="mask")
    st.gbs = const.tile([128, 3 * G3], dt.float16, tag="gbs")
    st.bhs = const.tile([128, 3 * 128], dt.float16, tag="bhs")
    st.sel4 = const.tile([128, FREE], dt.float16, tag="sel4")
    st.ones = const.tile([128, TC * BS], dt.float16, tag="ones")
    st.hbf = [const.tile([128, TC, FREE], dt.float16, tag=f"hbf{l}", name=f"hbf{l}")
              for l in range(3)]

    # ---- load weights / constants ------------------------------------
    for kc in range(KC0):
        nc.sync.dma_start(st.wih[0][:, kc, :], io["wihT0"][kc])
    for l in (1, 2):
        for kc in range(KC):
            nc.sync.dma_start(st.wih[l][:, kc, :], io[f"wihT{l}"][kc])
    for l in range(3):
        for kc in range(KC):
            nc.sync.dma_start(st.whhrz[l][:, kc, :], io[f"whhrzT{l}"][kc])
            nc.sync.dma_start(st.whhn[l][:, kc, :], io[f"whhnT{l}"][kc])
        nc.sync.dma_start(st.gbs[0:1, l * G3:(l + 1) * G3], io["gbstat"][l])
        nc.sync.dma_start(st.bhs[0:4, l * 128:(l + 1) * 128], io["bhhstat"][l])
    nc.sync.dma_start(st.sel4[0:4, :], io["sel4"])
    nc.sync.dma_start(st.ones[0:1, :], io["ones64"])

    # ---- prologue: mask, zero state ----------------------------------
    with ExitStack() as pctx:
        psb = pctx.enter_context(tc.tile_pool(name="psb", bufs=2))
        iota_sb = psb.tile([128, T], dt.float32, tag="iota")
        seq_sb = psb.tile([128, BS], dt.float32, tag="seq")
        nc.sync.dma_start(iota_sb[:], io["iota"])
        nc.sync.dma_start(seq_sb[:], io["seqrep"])
        for b in range(BS):
            nc.vector.tensor_scalar(st.mask[:, :, b], iota_sb[:],
                                    seq_sb[:, b:b + 1], None, op0=Alu.is_lt)

    for l in range(3):
        nc.gpsimd.memset(st.hbf[l][:, TC - 1, :], 0.0)

    # ---- wavefront rounds --------------------------------------------
    # Round r: (L0, chunk r), (L1, chunk r-1), (L2, chunk r-2).
    # chordT is host-rolled left by 2 chunks so that inside the hardware
    # loop (which runs L2's chunk offset i) L0's chunk i+2*TC sits at
    # offset i; peeled rounds 0/1 read chunks 0/1 at offsets 30/31*TC.
    def _csl(pos, lead):
        o = (pos + lead) * TC % T
        return slice(o, o + TC)

    def emit_round(r, pos):
        peeled = isinstance(pos, int)
        l0 = not peeled or 0 <= r < NCHUNK
        l1 = not peeled or 0 <= r - 1 < NCHUNK
        l2 = not peeled or 0 <= r - 2 < NCHUNK

        # stage previous layers' chunk outputs before they are overwritten
        if l1:
            src1 = stream.tile([128, TC, KC, BS], dt.float16, tag="src1")
            nc.sync.dma_start(src1[:], st.hbf[0][:])
        if l2:
            src2 = stream.tile([128, TC, KC, BS], dt.float16, tag="src2")
            nc.sync.dma_start(src2[:], st.hbf[1][:])
        if l0:
            src0 = stream.tile([128, TC, KC0, BS], dt.float16, tag="src0")
            tsl = _csl(r, -2) if peeled else bass.ds(pos, TC)
            for kc in range(KC0):
                nc.sync.dma_start(src0[:, :, kc, :], io["chordT"][kc, :, tsl, :])
        osl = None
        if l2:
            osl = _csl(r - 2, 0) if peeled else bass.ds(pos, TC)

        active = []
        if l0:
            active.append((0, src0, KC0))
        if l1:
            active.append((1, src1, KC))
        if l2:
            active.append((2, src2, KC))

        # -- gx GEMMs (m-groups round-robin across layers); r,z go to a
        # per-layer PSUM bank that the recurrence then accumulates onto,
        # n goes to SBUF; biases enter via K=1 matmuls with a ones row --
        przs, gxns = {}, {}
        for l, _, _ in active:
            przs[l] = prz_pool.tile([128, 2 * KC, TC, BS], dt.float32,
                                    tag="prz", name=f"prz{l}")
            gxns[l] = gxnp.tile([128, KC, TC, BS], dt.float32, tag="gxn",
                                name=f"gxn{l}")
        for m in range(MCH):
            for l, src, kcl in active:
                if m < 2 * KC:
                    tgt = przs[l][:, m, :, :]
                    first = (m == 0)   # one whole-bank clear per prz bank
                else:
                    pgx = pgx_pool.tile([128, PBANK], dt.float32, tag="pgx")
                    tgt = pgx[:, 0:TC * BS].rearrange("p (t b) -> p t b", t=TC)
                    first = True       # fresh bank per n m-group
                for kc in range(kcl):
                    nc.tensor.matmul(
                        tgt, st.wih[l][:, kc, m * 128:(m + 1) * 128],
                        src[:, :, kc, :],
                        start=(kc == 0 and first), stop=False,
                        skip_group_check=True)
                nc.tensor.matmul(
                    tgt, st.gbs[0:1, l * G3 + m * 128:l * G3 + (m + 1) * 128],
                    st.ones[0:1, :].rearrange("p (t b) -> p t b", t=TC),
                    start=False, stop=True, skip_group_check=True)
                if m >= 2 * KC:
                    nc.vector.tensor_copy(gxns[l][:, m - 2 * KC, :, :], tgt)

        if l2 and ABLATE != "nogates":
            mch = stream.tile([128, TC, BS], dt.float32, tag="maskch")
            nc.sync.dma_start(mch[:], st.mask[:, osl, :])
            osb = outp.tile([128, KC, TC, BS], dt.float32, tag="osb")

        # -- TC recurrence steps, round-robin across the active layers so
        # each layer's serial gate chain hides behind the other layers'
        # PE (LDWEIGHTS-bound) matmul bursts --
        inv = 1.0 / WSCALE
        for s in range(TC):
            sp = (s - 1) % TC          # previous step's h slot
            if ABLATE == "nochain":
                sp = TC - 1            # constant rhs: breaks serial chain
            pghns, rzs, rns, aNs, ns, ds = {}, {}, {}, {}, {}, {}

            def burst(l):
                prz, hbf = przs[l], st.hbf[l]
                whhrz, whhn = st.whhrz[l], st.whhn[l]
                pghn = pghn_pool.tile([128, PBANK], dt.float32, tag="pghn",
                                      name=f"pghn{l}")
                pghns[l] = pghn
                for m in range(2 * KC):
                    for kc in range(KC):
                        nc.tensor.matmul(
                            prz[:, m, s, :],
                            whhrz[:, kc, m * 128:(m + 1) * 128],
                            hbf[:, sp, kc * BS:(kc + 1) * BS],
                            start=False, stop=(kc == KC - 1),
                            skip_group_check=True)
                for m in range(KC):
                    for kc in range(KC):
                        nc.tensor.matmul(
                            pghn[:, m * BS:(m + 1) * BS],
                            whhn[:, kc, m * 128:(m + 1) * 128],
                            hbf[:, sp, kc * BS:(kc + 1) * BS],
                            start=(m == 0 and kc == 0), stop=False,
                            skip_group_check=True)
                nc.tensor.matmul(          # += b_hh_n via K=4 selector
                    pghn[:, 0:FREE].rearrange("p (h b) -> p h b", h=KC),
                    st.bhs[0:4, l * 128:(l + 1) * 128],
                    st.sel4[0:4, :].rearrange("p (h b) -> p h b", h=KC),
                    start=False, stop=True, skip_group_check=True)

            def sig(l):
                rzs[l] = tmp.tile([128, 2 * FREE], dt.float32, tag="rz",
                                  name=f"rz{l}")
                nc.scalar.activation(rzs[l][:], przs[l][:, :, s, :],
                                     Sigmoid, scale=inv)

            def rna(l):
                rns[l] = tmp.tile([128, FREE], dt.float32, tag="rn",
                                  name=f"rn{l}")
                nc.vector.tensor_mul(rns[l][:], pghns[l][:, 0:FREE],
                                     rzs[l][:, 0:FREE])
                aNs[l] = tmp.tile([128, FREE], dt.float32, tag="aN",
                                  name=f"aN{l}")
                nc.vector.tensor_add(aNs[l][:], rns[l][:], gxns[l][:, :, s, :])

            def tanh(l):
                ns[l] = tmp.tile([128, FREE], dt.float32, tag="n",
                                 name=f"n{l}")
                nc.scalar.activation(ns[l][:], aNs[l][:], Tanh, scale=inv)

            def hout(l):
                ds[l] = tmp.tile([128, FREE], dt.float32, tag="d",
                                 name=f"d{l}")
                nc.vector.tensor_sub(ds[l][:], st.hbf[l][:, sp, :], ns[l][:])
                zd = tmp.tile([128, FREE], dt.float32, tag="zd",
                              name=f"zd{l}")
                nc.vector.tensor_mul(zd[:], rzs[l][:, FREE:2 * FREE], ds[l][:])
                nc.vector.tensor_add(st.hbf[l][:, s, :], ns[l][:], zd[:])
                if l == 2:
                    nc.vector.tensor_mul(
                        osb[:, :, s, :],
                        st.hbf[2][:, s, :].rearrange("p (h b) -> p h b", h=KC),
                        mch[:, s:s + 1, :].broadcast_to([128, KC, BS]))

            # software-pipelined emission: each chain op lands in its
            # engine's FIFO at the earliest slot its inputs can be ready,
            # so no chain blocks another behind a not-yet-ready op
            acts = [a for a, _, _ in active]
            if ABLATE == "nogates":
                for l in acts:
                    burst(l)
                continue
            if len(acts) < 3:
                for l in acts:
                    burst(l)
                for f in (sig, rna, tanh, hout):
                    for l in acts:
                        f(l)
                continue
            for l in acts:
                burst(l)
            for f in (sig, rna, tanh, hout):
                for l in acts:
                    f(l)

        if l2 and ABLATE != "nogates":
            for hc in range(KC):
                nc.sync.dma_start(io["outT"][hc, :, osl, :],
                                  osb[:, hc, :, :])

    hint = (mybir.EngineType.PE,)
    # peeled fill rounds 0,1
    if PEEL:
        emit_round(0, 0)
        emit_round(1, 1)
    # steady-state rounds 2..31 as a hardware loop over L2's chunk offset
    if LOOPN:
        with tc.For_i(0, LOOPN * TC, TC, hint_engines=hint, name="main") as i:
            emit_round(None, i)
    # peeled drain rounds 32,33
    if PEEL:
        emit_round(NCHUNK, NCHUNK)
        emit_round(NCHUNK + 1, NCHUNK + 1)

    # optional timing-only repeat of the steady-state loop
    if RDEV:
        with tc.For_i(0, RDEV, 1, name="rep"):
            with tc.For_i(0, LOOPN * TC, TC, hint_engines=hint,
                          name="mainrep") as i:
                emit_round(None, i)
    ctx.close()


_CACHE = {}


def _get_program():
    if "nc" not in _CACHE:
        _CACHE["nc"] = _build_program()
    return _CACHE["nc"]


def _prep_shared(ws):
    """Host layout prep for the replicated weights (shared by all cores)."""
    sh = {}
    for l in range(3):
        w_ih, w_hh, b_ih, b_hh = ws[l]
        kcl = KC0 if l == 0 else KC
        sh[f"wihT{l}"] = np.ascontiguousarray(
            (w_ih.T * WSCALE).reshape(kcl, 128, G3)).astype(BF)
        whT = (w_hh.T * WSCALE).reshape(KC, 128, G3)
        sh[f"whhrzT{l}"] = np.ascontiguousarray(
            whT[:, :, :2 * H]).astype(WRZ_NP)
        sh[f"whhnT{l}"] = np.ascontiguousarray(whT[:, :, 2 * H:]).astype(BF)
    gbs = np.zeros((3, 1, G3), F32)
    bhs = np.zeros((3, 4, 128), F32)
    for l in range(3):
        _, _, b_ih, b_hh = ws[l]
        v = b_ih.copy()
        v[:2 * H] += b_hh[:2 * H]          # r,z gates absorb b_hh
        gbs[l, 0] = WSCALE * v
        bhs[l] = WSCALE * b_hh[2 * H:].reshape(4, 128)
    sh["gbstat"] = gbs.astype(BF)
    sh["bhhstat"] = bhs.astype(BF)
    sel = np.zeros((4, FREE), F32)
    for k in range(4):
        sel[k, k * BS:(k + 1) * BS] = 1.0
    sh["sel4"] = sel.astype(BF)
    sh["ones64"] = np.ones((1, TC * BS), BF)
    sh["iota"] = np.broadcast_to(
        np.arange(T, dtype=F32)[None, :], (128, T)).copy()
    return sh


def kernel(z, seq_lens, chord_embedding, fc_w, fc_b,
           w_ih0, w_hh0, b_ih0, b_hh0,
           w_ih1, w_hh1, b_ih1, b_hh1,
           w_ih2, w_hh2, b_ih2, b_hh2):
    z = np.asarray(z, F32)
    chord = np.asarray(chord_embedding, F32)
    seq = np.asarray(seq_lens)
    ws = [(np.asarray(w_ih0, F32), np.asarray(w_hh0, F32),
           np.asarray(b_ih0, F32), np.asarray(b_hh0, F32)),
          (np.asarray(w_ih1, F32), np.asarray(w_hh1, F32),
           np.asarray(b_ih1, F32), np.asarray(b_hh1, F32)),
          (np.asarray(w_ih2, F32), np.asarray(w_hh2, F32),
           np.asarray(b_ih2, F32), np.asarray(b_hh2, F32))]

    in_maps = _make_in_maps(z, seq, chord, np.asarray(fc_w, F32),
                            np.asarray(fc_b, F32), ws)
    res = _execute(in_maps)
    return _assemble(res.results)


def _make_in_maps(z, seq, chord, fc_w, fc_b, ws):
    sh = _prep_shared(ws)
    # fc layer is time-constant: fold it into the chord stream on host
    fc_h = np.maximum(z @ fc_w.T + fc_b, 0.0)          # [B, HID]
    chord_eff = fc_h[:, None, :] + chord / 100.0        # [B, T, HID]
    in_maps = []
    for c in range(NCORES):
        bs = slice(c * BS, (c + 1) * BS)
        m = dict(sh)
        chT = np.ascontiguousarray(
            chord_eff[bs].transpose(2, 1, 0)
            .reshape(KC0, 128, T, BS)).astype(BF)
        # roll time left by 2 chunks: chunk c sits at offset (c-2) % NCHUNK
        m["chordT"] = np.ascontiguousarray(np.roll(chT, -2 * TC, axis=2))
        m["seqrep"] = np.broadcast_to(
            seq[bs].astype(F32)[None, :], (128, BS)).copy()
        in_maps.append(m)
    return in_maps


def _execute(in_maps, **kw):
    nc = _get_program()
    return bass_utils.run_bass_kernel_spmd(nc, in_maps, list(range(NCORES)), **kw)


def _assemble(results):
    out = np.empty((B, T, H), F32)
    for c in range(NCORES):
        outT = np.asarray(results[c]["outT"])       # [KC,128,T,BS]
        out[c * BS:(c + 1) * BS] = (
            outT.transpose(3, 2, 0, 1).reshape(BS, T, H))
    return out
